# revision 10
# baseline (speedup 1.0000x reference)
"""Multi-head attention (B=4, S=2048, D=768, H=12) on 8 TRN2 NeuronCores.

Sharding: core i -> batch i//2, heads 6*(i%2) .. 6*(i%2)+6 (48 (b,h) pairs,
6 per core). Each core computes q^T/k^T in [d, s] layout, V in natural
[s, d] layout (bf16, with an appended ones-column so the softmax denominator
falls out of the attnV matmul), transposed scores S^T[k, q], exp on the
scalar engine (bf16 out), then the partial output projection over its 384
attention-output channels. The two cores sharing a batch have their partial
projections summed host-side, which stands in for the tensor-parallel
all-reduce.

Performance structure:
- Matmuls in f32r (TF32-like, 1 cyc/row at N>=256); paired score matmuls in
  64-row groups; attention inner loop software-pipelined (attnV trails the
  exp stream by TRAIL k-blocks; divisions of head pair j run inside pair
  j+1's loop) so the scalar engine's exp stream never stalls.
- x^T / w_qkv / w_v inputs in bf16: halves per-rep input DMA (9.8 -> 4.9 MB)
  and SBUF footprint; V/expS tiles bf16.
- Input tile pool hoisted out of the rep loop with bufs=2 so the next rep's
  input DMAs double-buffer under the current rep's attention phase.
- V-projection bias folded to the host (softmax rows sum to 1):
  y += b_v @ w_out + b_out in finish_output().
- Phase-B psum pool at bufs=8 (phase B has all banks to itself) so the next
  output block's projection matmuls overlap the previous block's bias-adds;
  output staging + y DMA in bf16 (halves output traffic, partials upcast
  to fp32 on the host before summing).
"""

import os

import numpy as np

import concourse.bass as bass
from concourse import bacc
import concourse.mybir as mybir
import concourse.tile as tile
from concourse.bass_utils import run_bass_kernel_spmd

F32 = mybir.dt.float32
F32R = mybir.dt.float32r
BF16 = mybir.dt.bfloat16
I16 = mybir.dt.int16
AF = mybir.ActivationFunctionType
ALU = mybir.AluOpType

B, S, D = 4, 2048, 768
H, HD = 12, 64
HPC = 6            # heads per core
GC = HPC * HD      # 384 channels per core
N_CORES = 8
SCALE = 1.0 / np.sqrt(np.float32(H))   # NOTE: reference scales by 1/sqrt(H)

# Schraudolph constants: bits = A_SCH * s_raw + B_SCH, int16 -> bitcast bf16
A_SCH = float(128.0 * np.log2(np.e) * SCALE)
B_SCH = float(128.0 * 127.0 - 7.5)

SPLIT_ATTNV = False
SPLIT_PROJ = False
SPLIT_OUTPROJ = False
# k-blocks whose second-head (hi==1) exp runs on the DVE (Schraudolph)
# instead of the scalar engine: unloads ACT (the phase-C co-critical engine)
# at ~1% extra attention-output error (validated in numpy; gate is 2e-2).
# Head A's exp always stays on ACT; kb 0-2 also stay on ACT so the DVE is
# clear of the div-carry burst (reciprocal) at the start of each pair.
DVE_KBS = frozenset(range(3, 16))
SKIP = frozenset()

_NC_CACHE = None
LAST_RESULTS = None


def _build(reps=1):
    nc = bacc.Bacc("TRN2", target_bir_lowering=False, debug=False,
                   num_devices=N_CORES)
    xt = nc.dram_tensor("xt", (D, S), BF16, kind="ExternalInput")
    wqk = nc.dram_tensor("wqk", (D, 2 * GC), BF16, kind="ExternalInput")
    wv = nc.dram_tensor("wv", (D, GC), BF16, kind="ExternalInput")
    bqk = nc.dram_tensor("bqk", (2 * GC,), F32, kind="ExternalInput")
    wout = nc.dram_tensor("wout", (GC, D), F32, kind="ExternalInput")
    y = nc.dram_tensor("y", (S, D), BF16, kind="ExternalOutput")

    NSB = S // 128        # 16 s-blocks
    NDS = D // 128        # 6 d-subtiles
    NOB = 2 * GC // 128   # 6 q+k output blocks
    NCS = GC // 128       # 3 c-subtiles for out-proj

    with tile.TileContext(nc) as tc:
        with (
            tc.tile_pool(name="const", bufs=1) as cpool,
            tc.tile_pool(name="bigqv", bufs=1) as bigqv,
            tc.tile_pool(name="bigd", bufs=1) as bigd,
        ):
            ones_sb = cpool.tile([1, 128], F32)
            nc.gpsimd.memset(ones_sb[:], 1.0)
            ones_r = cpool.tile([1, 128], F32R)
            nc.vector.tensor_copy(ones_r[:], ones_sb[:])
            # preload the exp ACT table during phase B instead of on the
            # critical first softmax tile
            warm = cpool.tile([1, 8], F32)
            nc.scalar.activation(warm[:], ones_sb[:, :8], AF.Exp)
            bqk_sb = cpool.tile([128, NOB], F32)
            nc.sync.dma_start(bqk_sb[:], bqk.ap().rearrange("(ob p) -> p ob", p=128))
            wout_sb = cpool.tile([128, NCS, D], F32R)

            qkT = bigqv.tile([128, NOB, S], F32R)     # blocks 0-2 q^T, 3-5 k^T
            V_sb = bigqv.tile([128, NSB, HPC * (HD + 1)], BF16)  # V + ones col
            attnT = bigd.tile([128, NCS, S], F32R)    # attention out, [c, s]

            V_view = V_sb[:].rearrange("p b (h e) -> p b h e", e=HD + 1)
            ones_col = cpool.tile([128, 1], BF16)
            nc.gpsimd.memset(ones_col[:], 1.0)
            nc.vector.tensor_copy(
                V_view[:, :, :, HD], ones_col[:, :, None].to_broadcast([128, NSB, HPC])
            )

            xt_src = xt.ap().rearrange("(ds p) s -> p ds s", p=128)
            wqk_src = wqk.ap().rearrange("(ds p) o -> p ds o", p=128)

            rep_ctx = tc.tile_pool(name="xtp", bufs=2)
            xtp = rep_ctx.__enter__()
            for _rep in range(reps):
                # ---- Phase B: projections ----
                OB_ORDER = (0, 3, 1, 4, 2, 5)
                with (
                    tc.tile_pool(name="psb", bufs=8, space="PSUM") as psb,
                ):
                    xt_sb = xtp.tile([128, NDS, S], BF16, tag="xt",
                                     name="xt_sb")
                    wqk_sb = xtp.tile([128, NDS, 2 * GC], BF16, tag="wq",
                                      name="wqk_sb")
                    wv_sb = xtp.tile([128, NDS, GC], BF16, tag="wv",
                                     name="wv_sb")
                    for ob in (0, 3):
                        nc.sync.dma_start(
                            wqk_sb[:, :, ob * 128:(ob + 1) * 128],
                            wqk_src[:, :, ob * 128:(ob + 1) * 128],
                        )
                    for sc in range(4):
                        nc.sync.dma_start(
                            xt_sb[:, :, sc * 512:(sc + 1) * 512],
                            xt_src[:, :, sc * 512:(sc + 1) * 512],
                        )
                    for ob in (1, 4, 2, 5):
                        nc.sync.dma_start(
                            wqk_sb[:, :, ob * 128:(ob + 1) * 128],
                            wqk_src[:, :, ob * 128:(ob + 1) * 128],
                        )
                    nc.sync.dma_start(
                        wv_sb[:],
                        wv.ap().rearrange("(ds p) o -> p ds o", p=128),
                    )
                    if _rep == 0:
                        nc.sync.dma_start(
                            wout_sb[:],
                            wout.ap().rearrange(
                                "(cs p) o -> p cs o", p=128).bitcast(F32R),
                        )

                    # q^T / k^T: [o, s] = wqk^T @ x^T
                    halves = ((0, 64), (64, 128)) if SPLIT_PROJ else ((0, 128),)
                    for ob in OB_ORDER:
                        pss4 = [psb.tile([128, 512], F32, tag="ps",
                                         name=f"ps{ob}_{sc}") for sc in range(4)]
                        for ds in range(NDS) if "proj" not in SKIP else ():
                            for sc in range(4):
                                for hi, (r0, r1) in enumerate(halves):
                                    nc.tensor.matmul(
                                        pss4[sc][:],
                                        wqk_sb[r0:r1, ds, ob * 128:(ob + 1) * 128],
                                        xt_sb[r0:r1, ds, sc * 512:(sc + 1) * 512],
                                        start=(ds == 0 and hi == 0),
                                        stop=(ds == NDS - 1
                                              and hi == len(halves) - 1),
                                        skip_group_check=SPLIT_PROJ,
                                    )
                        for sc in range(4):
                            nc.vector.tensor_scalar_add(
                                qkT[:, ob, sc * 512:(sc + 1) * 512], pss4[sc][:],
                                bqk_sb[:, ob:ob + 1],
                            )

                    # V natural: [s, o] = x @ wv   (bias folded to host)
                    for sb in range(NSB):
                        ps = psb.tile([128, 512], F32, tag="ps")
                        for ds in range(NDS) if "proj" not in SKIP else ():
                            for hi, (r0, r1) in enumerate(halves):
                                nc.tensor.matmul(
                                    ps[:, :GC],
                                    xt_sb[r0:r1, ds, sb * 128:(sb + 1) * 128],
                                    wv_sb[r0:r1, ds, :],
                                    start=(ds == 0 and hi == 0),
                                    stop=(ds == NDS - 1
                                          and hi == len(halves) - 1),
                                    skip_group_check=True,
                                )
                        nc.vector.tensor_copy(V_view[:, sb, :, 0:HD], ps[:, :GC])

                # ---- Phase C: attention per (head-pair, q-half) ----
                with (
                    tc.tile_pool(name="bigc", bufs=1) as bigc,
                    tc.tile_pool(name="cw", bufs=1) as cw,
                    tc.tile_pool(name="pss", bufs=2, space="PSUM") as pss,
                    tc.tile_pool(name="pso", bufs=2, space="PSUM") as pso,
                ):
                    QH = S // 2  # 1024
                    NBUF = 8     # rotating S^T exp slots (2 per k-block)
                    expS = bigc.tile([128, NBUF, QH], BF16)

                    def slot(kb, hi):
                        return (2 * kb + hi) % NBUF

                    def make_div(ps_o, h, qh):
                        # normalize out'[d, q] by Z[q] (ones-column row).
                        # reciprocal [1,QH] (DVE, from psum), partition-
                        # broadcast (gpsimd), multiply (DVE): no PE matmul,
                        # no shared-psum-pool collision with the score tiles.
                        # Split into two stages so both heads' reciprocals and
                        # broadcasts issue before either multiply (in-order
                        # engine queues), letting ps_o free as early as
                        # possible for the next pair's attnV.
                        if "div" in SKIP:
                            return lambda: None, lambda: None
                        base = (h % 2) * 64
                        qob = h // 2
                        rz = cw.tile([1, QH], F32, tag="rz", bufs=2, name="rz")
                        rzb_sb = cw.tile([64, QH], F32, tag="rzb", bufs=2,
                                         name="rzb_sb")

                        def recip_bcast():
                            with nc.allow_low_precision(reason="f32r softmax denom"):
                                nc.vector.reciprocal(rz[:], ps_o[HD:HD + 1, :])
                            nc.gpsimd.partition_broadcast(rzb_sb[:], rz[:])

                        def norm():
                            nc.vector.tensor_mul(
                                attnT[base:base + 64, qob, qh * QH:(qh + 1) * QH],
                                ps_o[0:HD, :], rzb_sb[:],
                            )
                        return recip_bcast, norm

                    vhalves = ((0, 64), (64, 128)) if SPLIT_ATTNV else ((0, 128),)

                    def attn_v(ps_o, h, kb, start, stop):
                        if "attnv" in SKIP:
                            return
                        sl = slot(kb, h % 2)
                        for qc in range(2):
                            for hi, (r0, r1) in enumerate(vhalves):
                                nc.tensor.matmul(
                                    ps_o[:, qc * 512:(qc + 1) * 512],
                                    V_sb[r0:r1, kb,
                                         h * (HD + 1):(h + 1) * (HD + 1)],
                                    expS[r0:r1, sl,
                                         qc * 512:(qc + 1) * 512],
                                    start=(start and hi == 0),
                                    stop=(stop and hi == len(vhalves) - 1),
                                    skip_group_check=True,
                                )

                    def do_exp(ps_s, kb, hi):
                        if "exp" in SKIP:
                            return
                        sl = slot(kb, hi)
                        if hi == 1 and kb in DVE_KBS:
                            nc.vector.tensor_scalar(
                                expS[:, sl, :].bitcast(I16), ps_s[:],
                                A_SCH, B_SCH, ALU.mult, ALU.add,
                            )
                        else:
                            nc.scalar.activation(
                                expS[:, sl, :], ps_s[:], AF.Exp,
                                scale=float(SCALE),
                            )

                    TRAIL = 3
                    carry = []   # closures from the previous (pair, qh)
                    for hp in range(HPC // 2):
                        hA, hB = 2 * hp, 2 * hp + 1
                        qob = hp
                        kob = NCS + hp
                        for qh in range(2):
                            ps_oA = pso.tile([HD + 1, QH], F32, tag="o", name="ps_oA")
                            ps_oB = pso.tile([HD + 1, QH], F32, tag="o", name="ps_oB")
                            for kb in range(NSB):
                                ps_sA = pss.tile([128, QH], F32, tag="s", name="ps_sA")
                                ps_sB = pss.tile([128, QH], F32, tag="s", name="ps_sB")
                                for base, ps_s in ((0, ps_sA), (64, ps_sB)) \
                                        if "scores" not in SKIP else ():
                                    for qc in range(2):
                                        nc.tensor.matmul(
                                            ps_s[:, qc * 512:(qc + 1) * 512],
                                            qkT[base:base + 64, kob,
                                                kb * 128:(kb + 1) * 128],
                                            qkT[base:base + 64, qob,
                                                qh * QH + qc * 512:
                                                qh * QH + (qc + 1) * 512],
                                            start=True, stop=True,
                                        )
                                do_exp(ps_sA, kb, 0)
                                do_exp(ps_sB, kb, 1)
                                if kb < len(carry):
                                    carry[kb]()
                                if kb >= TRAIL:
                                    pk = kb - TRAIL
                                    attn_v(ps_oA, hA, pk, start=(pk == 0), stop=False)
                                    attn_v(ps_oB, hB, pk, start=(pk == 0), stop=False)
                            for pk in range(NSB - TRAIL, NSB - 1):
                                attn_v(ps_oA, hA, pk, start=False, stop=False)
                                attn_v(ps_oB, hB, pk, start=False, stop=False)
                            rbA, normA = make_div(ps_oA, hA, qh)
                            rbB, normB = make_div(ps_oB, hB, qh)
                            carry = [
                                lambda a=ps_oA, b=ps_oB, h1=hA, h2=hB, \
                                        rA=rbA, rB=rbB: (
                                    attn_v(a, h1, NSB - 1, start=False, stop=True),
                                    attn_v(b, h2, NSB - 1, start=False, stop=True),
                                    rA(), rB(),
                                ),
                                lambda nA=normA, nB=normB: (nA(), nB()),
                            ]
                    for f in carry:
                        f()

                    # ---- Phase D: output projection (partial, 384 c) ----
                    chalves = ((0, 64), (64, 128)) if SPLIT_OUTPROJ else ((0, 128),)
                    for sb in range(NSB):
                        ps_f = pss.tile([128, QH], F32, tag="s", name="ps_f")
                        for cs in range(NCS) if "outproj" not in SKIP else ():
                            for hi, (r0, r1) in enumerate(chalves):
                                for o0, n in ((0, 512), (512, 256)):
                                    nc.tensor.matmul(
                                        ps_f[:, o0:o0 + n],
                                        attnT[r0:r1, cs, sb * 128:(sb + 1) * 128],
                                        wout_sb[r0:r1, cs, o0:o0 + n],
                                        start=(cs == 0 and hi == 0),
                                        stop=(cs == NCS - 1
                                              and hi == len(chalves) - 1),
                                        skip_group_check=SPLIT_OUTPROJ,
                                    )
                        ostage = cw.tile([128, D], BF16, tag="ostage", bufs=3,
                                         name="ostage")
                        nc.any.tensor_copy(ostage[:], ps_f[:, :D])
                        nc.sync.dma_start(y.ap()[sb * 128:(sb + 1) * 128, :], ostage[:])
            rep_ctx.__exit__(None, None, None)

    nc.compile()
    return nc


def _get_nc():
    global _NC_CACHE
    if _NC_CACHE is None:
        _NC_CACHE = _build()
    return _NC_CACHE


def make_in_maps(x, w_qkv, b_qkv, w_out, b_out):
    x = np.asarray(x, dtype=np.float32)
    w_qkv = np.asarray(w_qkv, dtype=np.float32)
    b_qkv = np.asarray(b_qkv, dtype=np.float32)
    w_out = np.asarray(w_out, dtype=np.float32)

    in_maps = []
    for i in range(N_CORES):
        b = i // 2
        c0 = (i % 2) * GC
        q_sl = slice(c0, c0 + GC)
        k_sl = slice(D + c0, D + c0 + GC)
        v_sl = slice(2 * D + c0, 2 * D + c0 + GC)
        import ml_dtypes
        bf = ml_dtypes.bfloat16
        in_maps.append({
            "xt": np.ascontiguousarray(x[b].T.astype(bf)),
            "wqk": np.ascontiguousarray(np.concatenate(
                [w_qkv[:, q_sl], w_qkv[:, k_sl]], axis=1).astype(bf)),
            "wv": np.ascontiguousarray(w_qkv[:, v_sl].astype(bf)),
            "bqk": np.ascontiguousarray(
                np.concatenate([b_qkv[q_sl], b_qkv[k_sl]])),
            "wout": np.ascontiguousarray(w_out[c0:c0 + GC, :]),
        })
    return in_maps


def finish_output(res, x, w_qkv, b_qkv, w_out, b_out):
    b_qkv = np.asarray(b_qkv, dtype=np.float32)
    w_out = np.asarray(w_out, dtype=np.float32)
    b_out = np.asarray(b_out, dtype=np.float32)
    # V-projection bias passes through the softmax average; fold it into the
    # output bias: y += b_v @ w_out + b_out
    b_eff = b_qkv[2 * D:] @ w_out + b_out
    out = np.empty((B, S, D), dtype=np.float32)
    for b in range(B):
        out[b] = (res.results[2 * b]["y"].astype(np.float32)
                  + res.results[2 * b + 1]["y"].astype(np.float32) + b_eff)
    return out


def kernel(x, w_qkv, b_qkv, w_out, b_out):
    global LAST_RESULTS
    in_maps = make_in_maps(x, w_qkv, b_qkv, w_out, b_out)
    nc = _get_nc()
    res = run_bass_kernel_spmd(nc, in_maps, core_ids=list(range(N_CORES)))
    LAST_RESULTS = res
    return finish_output(res, x, w_qkv, b_qkv, w_out, b_out)

